# revision 10
# baseline (speedup 1.0000x reference)
"""MultiHeadSelection Trainium2 kernel.

scores[b,i,j,p] = sum_k tanh(x[b,i]@u_a[:,k] + x[b,j]@w_a[:,k] + b_s[k]) * v[k,p]

Shapes (hardcoded): x [8,256,768], u_a/w_a [768,256], b_s [256], v [256,50]
-> out [8,256,256,50] float32.

Sharding: data-parallel over batch, one batch element per NeuronCore (8 cores).
Each core:
  stage 1 (tiny): left_T[k,i] = (x_b @ u_a)^T, right_T[k,j] = (x_b @ w_a)^T
                  via PE matmuls with k on psum partitions (lhsT = weights
                  [h,k] chunk, rhs = x^T [h,*]); bias_all = left_T + b_s.
  stage 2 (hot):  for each i: pre[k,j] = right_T[k,j] + bias_all[k,i]
                  (DVE tensor_scalar, bf16 4x mode), tanh on ACT in big
                  FD=IB*256 ops, then PE matmuls lhsT=tanh[k, j-chunk]
                  (128-col bf16 weights -> FWL), rhs = v[k-chunk] bf16,
                  accumulated over the 2 k-chunks into psum [j_local, p].
                  psum -> SBUF staging (DVE) -> one 819KB DMA per 16-i block.
"""

import numpy as np
from contextlib import ExitStack

import concourse.bass as bass
import concourse.mybir as mybir
import concourse.tile as tile
from concourse import bacc

B, S, H, K, P = 8, 256, 768, 256, 50
NCORES = 8
IB = 16            # i-block size (ACT op free dim = IB*S = 4096)
GRP = 4            # i's per psum output tile ([128, GRP*2*50] = 1600B/bank)
KC = K // 128      # 2 k-chunks
HC = H // 128      # 6 h-chunks
JC = S // 128      # 2 j-chunks

F32 = mybir.dt.float32
BF16 = mybir.dt.bfloat16


def _build_nc():
    # Bacc (not raw Bass): its compile() pass splits multi-semaphore waits
    # into EventSemaphore instructions — TRN2 engine instructions hold 1 wait.
    nc = bacc.Bacc("TRN2", target_bir_lowering=False, debug=False,
                   enable_partition_id=False)

    xb = nc.dram_tensor("xb", [S, H], F32, kind="ExternalInput").ap()
    ua = nc.dram_tensor("ua", [H, K], F32, kind="ExternalInput").ap()
    wa = nc.dram_tensor("wa", [H, K], F32, kind="ExternalInput").ap()
    bs = nc.dram_tensor("bs", [K], F32, kind="ExternalInput").ap()
    vv = nc.dram_tensor("vv", [K, P], F32, kind="ExternalInput").ap()
    sc = nc.dram_tensor("scores", [S, S, P], F32, kind="ExternalOutput").ap()

    with ExitStack() as ctx:
        tc = ctx.enter_context(tile.TileContext(nc))
        singles = ctx.enter_context(tc.tile_pool(name="singles", bufs=1))
        work = ctx.enter_context(tc.tile_pool(name="work", bufs=2))
        outp = ctx.enter_context(tc.tile_pool(name="outp", bufs=2))

        # ---- constants ----
        v_bf = singles.tile([128, KC, P], BF16)
        for kc in range(KC):
            nc.gpsimd.dma_start(out=v_bf[:, kc, :], in_=vv[kc * 128:(kc + 1) * 128, :])
        bs_dma = singles.tile([128, KC], F32)
        for kc in range(KC):
            nc.sync.dma_start(out=bs_dma[:, kc:kc + 1], in_=bs[kc * 128:(kc + 1) * 128])
        # Bounce through a DVE copy so the DMA-completion wait lands on the
        # copy, not on the single-wait-slot TensorScalarPtr that consumes it.
        bs_col = singles.tile([128, KC], F32)
        nc.vector.tensor_copy(out=bs_col, in_=bs_dma)

        r_bf = singles.tile([128, KC, S], BF16)       # right_T, bf16
        bias_all = singles.tile([128, KC, S], F32)    # left_T + b_s, fp32

        # ---- stage 1 ----
        with tc.tile_pool(name="s1", bufs=1) as s1, \
             tc.tile_pool(name="s1d", bufs=1, space="DRAM") as s1d, \
             tc.tile_pool(name="ps1", bufs=2, space="PSUM") as ps1:
            u_bf = s1.tile([128, HC, K], BF16)
            w_bf = s1.tile([128, HC, K], BF16)
            for hc in range(HC):
                nc.gpsimd.dma_start(out=u_bf[:, hc, :], in_=ua[hc * 128:(hc + 1) * 128, :])
                nc.gpsimd.dma_start(out=w_bf[:, hc, :], in_=wa[hc * 128:(hc + 1) * 128, :])

            # x -> bf16 (DRAM scratch) -> transposed into SBUF as [h, i]
            xd = s1d.tile([S, H], BF16)
            nc.gpsimd.dma_start(out=xd, in_=xb)  # fp32 -> bf16 cast in DMA
            x_T = s1.tile([128, HC, S], BF16)
            for hc in range(HC):
                nc.sync.dma_start_transpose(out=x_T[:, hc, :], in_=xd[:, hc * 128:(hc + 1) * 128])

            for kc in range(KC):
                ps_r = ps1.tile([128, S], F32, tag="ps_r")
                ps_l = ps1.tile([128, S], F32, tag="ps_l")
                for hc in range(HC):
                    nc.tensor.matmul(ps_r, lhsT=w_bf[:, hc, kc * 128:(kc + 1) * 128],
                                     rhs=x_T[:, hc, :], start=(hc == 0), stop=(hc == HC - 1))
                for hc in range(HC):
                    nc.tensor.matmul(ps_l, lhsT=u_bf[:, hc, kc * 128:(kc + 1) * 128],
                                     rhs=x_T[:, hc, :], start=(hc == 0), stop=(hc == HC - 1))
                nc.vector.tensor_copy(out=r_bf[:, kc, :], in_=ps_r)
                # Two-step (copy then add) keeps the TensorScalarPtr at a
                # single semaphore wait: its ISA encoding has only one wait
                # slot, and a direct PSUM read would need PE + DMA waits.
                lt = s1.tile([128, S], F32, tag="lt")
                nc.vector.tensor_copy(out=lt, in_=ps_l)
                nc.vector.tensor_scalar_add(out=bias_all[:, kc, :], in0=lt,
                                            scalar1=bs_col[:, kc:kc + 1])

        # ---- stage 2 ----
        pso = ctx.enter_context(tc.tile_pool(name="pso", bufs=6, space="PSUM"))
        for blk in range(S // IB):
            pre = work.tile([128, KC, IB, S], BF16, tag="pre")
            th = work.tile([128, KC, IB, S], BF16, tag="th")
            # Absorb the buffer-reuse (WAR vs ACT) semaphore waits into this
            # memset: the TensorScalarPtr ISA struct has only one sync-wait
            # slot, so the preadds below must not carry cross-engine waits.
            nc.vector.memset(pre[:, 0, 0, 0:2], 0.0)
            for kc in range(KC):
                for il in range(IB):
                    i = blk * IB + il
                    nc.vector.tensor_scalar_add(out=pre[:, kc, il, :], in0=r_bf[:, kc, :],
                                                scalar1=bias_all[:, kc, i:i + 1])
                nc.scalar.activation(out=th[:, kc], in_=pre[:, kc],
                                     func=mybir.ActivationFunctionType.Tanh)
            ost = outp.tile([128, IB, JC, P], F32, tag="ost")
            for g in range(IB // GRP):
                po = pso.tile([128, GRP, JC, P], F32, tag="po")
                for gi in range(GRP):
                    il = g * GRP + gi
                    for jc in range(JC):
                        for kc in range(KC):
                            nc.tensor.matmul(po[:, gi, jc, :],
                                             lhsT=th[:, kc, il, jc * 128:(jc + 1) * 128],
                                             rhs=v_bf[:, kc, :],
                                             start=(kc == 0), stop=(kc == KC - 1))
                nc.vector.tensor_copy(out=ost[:, g * GRP:(g + 1) * GRP], in_=po)
            oap = sc[blk * IB:(blk + 1) * IB].rearrange("i (jc jl) p -> jl i jc p", jc=JC)
            nc.sync.dma_start(out=oap, in_=ost)

    return nc


_RUNNER = None


def _get_runner():
    global _RUNNER
    if _RUNNER is not None:
        return _RUNNER
    import jax
    from jax.sharding import Mesh, PartitionSpec
    from jax.experimental.shard_map import shard_map
    from concourse.bass2jax import install_neuronx_cc_hook, _bass_exec_p

    install_neuronx_cc_hook()
    nc = _build_nc()
    if not nc.is_finalized():
        nc.finalize()

    in_names, out_names, out_avals = [], [], []
    for alloc in nc.m.functions[0].allocations:
        if not isinstance(alloc, mybir.MemoryLocationSet):
            continue
        if alloc.kind not in ("ExternalInput", "ExternalOutput"):
            continue
        name = alloc.memorylocations[0].name
        if alloc.kind == "ExternalInput":
            in_names.append(name)
        else:
            out_names.append(name)
            out_avals.append(jax.core.ShapedArray(tuple(alloc.tensor_shape),
                                                  mybir.dt.np(alloc.dtype)))
    n_params = len(in_names)
    all_in_names = tuple(in_names + out_names)

    def _body(*args):
        outs = _bass_exec_p.bind(
            *args,
            out_avals=tuple(out_avals),
            in_names=all_in_names,
            out_names=tuple(out_names),
            lowering_input_output_aliases=(),
            sim_require_finite=True,
            sim_require_nnan=True,
            nc=nc,
        )
        return tuple(outs)

    devices = jax.devices()[:NCORES]
    assert len(devices) == NCORES, f"need {NCORES} cores, got {len(devices)}"
    mesh = Mesh(np.asarray(devices), ("core",))
    nin = n_params + len(out_names)
    fn = jax.jit(
        shard_map(_body, mesh=mesh,
                  in_specs=(PartitionSpec("core"),) * nin,
                  out_specs=(PartitionSpec("core"),) * len(out_names),
                  check_rep=False),
        keep_unused=True,
    )
    _RUNNER = (fn, in_names, out_names, out_avals, mesh)
    return _RUNNER


def _concat_args(x, u_a, w_a, b_s, v, in_names, out_avals):
    x = np.ascontiguousarray(np.asarray(x, dtype=np.float32))
    u_a = np.asarray(u_a, dtype=np.float32)
    w_a = np.asarray(w_a, dtype=np.float32)
    b_s = np.asarray(b_s, dtype=np.float32)
    v = np.asarray(v, dtype=np.float32)
    per = {
        "xb": x.reshape(NCORES * S, H),
        "ua": np.tile(u_a, (NCORES, 1)),
        "wa": np.tile(w_a, (NCORES, 1)),
        "bs": np.tile(b_s, NCORES),
        "vv": np.tile(v, (NCORES, 1)),
    }
    args = [per[n] for n in in_names]
    args += [np.zeros((NCORES * a.shape[0], *a.shape[1:]), a.dtype) for a in out_avals]
    return args


def kernel(x, u_a, w_a, b_s, v):
    fn, in_names, out_names, out_avals, mesh = _get_runner()
    args = _concat_args(x, u_a, w_a, b_s, v, in_names, out_avals)
    outs = fn(*args)
    scores = np.asarray(outs[out_names.index("scores")])
    return scores.reshape(B, S, S, P)


def bench(x, u_a, w_a, b_s, v, iters=20):
    """Time the on-device execution (inputs pre-staged on device).

    Returns (avg_seconds_per_iter, outputs_of_last_iter_as_np)."""
    import time
    import jax
    from jax.sharding import NamedSharding, PartitionSpec

    fn, in_names, out_names, out_avals, mesh = _get_runner()
    args = _concat_args(x, u_a, w_a, b_s, v, in_names, out_avals)
    sh = NamedSharding(mesh, PartitionSpec("core"))
    dargs = [jax.device_put(a, sh) for a in args]
    # warmup (also triggers compile)
    outs = fn(*dargs)
    jax.block_until_ready(outs)
    t0 = time.perf_counter()
    for _ in range(iters):
        outs = fn(*dargs)
    jax.block_until_ready(outs)
    t1 = time.perf_counter()
    return (t1 - t0) / iters, outs
